# revision 1
# baseline (speedup 1.0000x reference)
"""Multi-head causal attention (B=4, T=2048, N=1024, H=16) on 8 TRN2 NeuronCores.

Sharding: core c = (batch b = c//2, head-group hg = c%2). Each core computes
full-T causal attention for its 8 heads of its batch, plus the partial output
projection for its head rows. Host sums the two head-group partials per batch
and adds b_proj (and the v-bias contribution, which is exact because softmax
rows sum to 1).

Per-core kernel (all big matmuls in f32r = full-rate 4-byte dtype):
  phase 1: qkT[j,t] = Wqk_shard.T @ x.T and v[t,d] = x @ Wv_shard; v is stored
           bf16 with a ones-column interleaved per head ([64 v | 1] x 8).
  phase 2: scores are computed TRANSPOSED, [k-part, q-free], one matmul per
           (128-k-block, 512-q-chunk, head): lhsT = kT block [64,128],
           rhs = qT chunk [64,512]. exp (ACT, scale=1/8) writes attT
           directly to SBUF bf16; causal masking is a bf16 multiply on the
           <=4 diagonal blocks. The av matmul lhsT = v[128,65] (64 v cols +
           ones), rhs = attT [128,512] accumulates y.T AND the softmax row
           sums into PSUM [65, 512] in one chain. Normalize: reciprocal of
           the sums row, gpsimd partition-broadcast, DVE multiply into the
           projection-ready yT tile [128 headpair-dims, 512 q]. Projection:
           yT.T @ Wp_shard in f32r.
"""
import numpy as np
import ml_dtypes
from contextlib import ExitStack

import concourse.bass as bass
import concourse.mybir as mybir
from concourse import bacc
from concourse import bass_utils as _bu
from concourse.bass_utils import run_bass_kernel_spmd
from concourse.tile import TileContext



F32 = mybir.dt.float32
F32R = mybir.dt.float32r
BF16 = mybir.dt.bfloat16
AF = mybir.ActivationFunctionType

B, T, N, H = 4, 2048, 1024, 16
Dh = 64
HG = 512            # head-group width per core (8 heads x 64)
NCORES = 8
KT = N // 128       # 8 contraction tiles for qkv
NQC = T // 512      # 4 q-chunks
NVT = T // 128      # 16 v tiles / k blocks

_CACHE = {}
_last_results = None


def _build():
    if "nc" in _CACHE:
        return _CACHE["nc"]

    nc = bacc.Bacc("TRN2", target_bir_lowering=False)

    xT = nc.declare_dram_parameter("xT", [N, T], BF16, isOutput=False)
    wqk = nc.declare_dram_parameter("wqk", [N, 2 * HG], BF16, isOutput=False)
    wv = nc.declare_dram_parameter("wv", [N, HG], BF16, isOutput=False)
    wp = nc.declare_dram_parameter("wp", [HG, N], F32R, isOutput=False)
    bqk = nc.declare_dram_parameter("bqk", [2 * HG, 1], F32, isOutput=False)
    maskt = nc.declare_dram_parameter("maskt", [4, 128, 512], BF16,
                                      isOutput=False)
    out = nc.declare_dram_parameter("out", [T, N], F32, isOutput=True)

    with TileContext(nc) as tc, ExitStack() as outer:
        consts = outer.enter_context(tc.tile_pool(name="consts", bufs=1))
        resid = outer.enter_context(tc.tile_pool(name="resid", bufs=1))

        mt_sb = []
        for m in range(4):
            t = consts.tile([128, 512], BF16, name=f"mt_sb{m}", tag=f"mt{m}")
            nc.sync.dma_start(out=t, in_=maskt[m, :, :])
            mt_sb.append(t)
        bqk_sb = []
        for j in range(8):
            t = consts.tile([128, 1], F32, name=f"bqk_sb{j}", tag=f"bqk{j}")
            nc.sync.dma_start(out=t, in_=bqk[j * 128:(j + 1) * 128, :])
            bqk_sb.append(t)

        # residents: qT/kT per head-pair tile [128, T]; v per 128-row chunk,
        # bf16, layout [64 v-cols + ones] x 8 heads = 520 cols
        qT = [resid.tile([128, T], BF16, name=f"qT{j}", tag=f"qT{j}")
              for j in range(4)]
        kTt = [resid.tile([128, T], BF16, name=f"kT{j}", tag=f"kT{j}")
               for j in range(4)]
        vt = [resid.tile([128, 8, 65], BF16, name=f"v{m}", tag=f"v{m}")
              for m in range(NVT)]

        # ---------------- phase 1: qkv projection ----------------
        with ExitStack() as p1:
            wpool = p1.enter_context(tc.tile_pool(name="wpool", bufs=1))
            xpool = p1.enter_context(tc.tile_pool(name="xpool", bufs=16))
            qkps = p1.enter_context(tc.tile_pool(name="qkps", bufs=3,
                                                 space="PSUM"))
            vps = p1.enter_context(tc.tile_pool(name="vps", bufs=2,
                                                space="PSUM"))

            wqk_t = []
            wv_t = []
            xt0 = []
            for k in range(KT):
                tx = xpool.tile([128, 512], BF16, name=f"xt0_{k}", tag="xt")
                nc.sync.dma_start(out=tx, in_=xT[k * 128:(k + 1) * 128,
                                                 0:512])
                xt0.append(tx)
                t = wpool.tile([128, HG], BF16, name=f"wv{k}", tag=f"wv{k}")
                nc.sync.dma_start(out=t, in_=wv[k * 128:(k + 1) * 128, :])
                wv_t.append(t)
            for k in range(KT):
                t = wpool.tile([128, 2 * HG], BF16, name=f"wqk{k}",
                               tag=f"wqk{k}")
                nc.sync.dma_start(out=t, in_=wqk[k * 128:(k + 1) * 128, :])
                wqk_t.append(t)

            for tck in range(4):          # t-chunks of 512
                c0 = tck * 512
                if tck == 0:
                    xt = xt0
                else:
                    xt = []
                    for k in range(KT):
                        t = xpool.tile([128, 512], BF16,
                                       name=f"xt{tck}_{k}", tag="xt")
                        nc.sync.dma_start(out=t,
                                          in_=xT[k * 128:(k + 1) * 128,
                                                 c0:c0 + 512])
                        xt.append(t)
                # v [t, d] : lhsT = xT tile cols, rhs = wv
                for mc in range(4):
                    ps = vps.tile([128, HG], F32, name=f"v_ps{tck}_{mc}",
                                  tag="v")
                    for k in range(KT):
                        nc.tensor.matmul(
                            ps,
                            xt[k][:, mc * 128:(mc + 1) * 128],
                            wv_t[k],
                            start=(k == 0), stop=(k == KT - 1))
                    dst = vt[tck * 4 + mc]
                    nc.vector.tensor_copy(
                        dst[:, :, 0:64],
                        ps.rearrange("p (h c) -> p h c", c=64))
                    nc.vector.memset(dst[:, :, 64:65], 1.0)
                # qkT [j, t] : lhsT = wqk tile cols, rhs = xT
                for jc in range(8):
                    ps = qkps.tile([128, 512], F32, name=f"qk_ps{tck}_{jc}",
                                   tag="qk")
                    for k in range(KT):
                        nc.tensor.matmul(
                            ps,
                            wqk_t[k][:, jc * 128:(jc + 1) * 128],
                            xt[k],
                            start=(k == 0), stop=(k == KT - 1))
                    dst = (qT[jc] if jc < 4 else kTt[jc - 4])
                    nc.scalar.add(dst[:, c0:c0 + 512], ps, bqk_sb[jc])

        # ---------------- phase 2: attention + projection ----------------
        with ExitStack() as p2:
            wppool = p2.enter_context(tc.tile_pool(name="wppool", bufs=1))
            attTp = p2.enter_context(tc.tile_pool(name="attTp", bufs=8))
            smallp = p2.enter_context(tc.tile_pool(name="smallp", bufs=4))
            bcp = p2.enter_context(tc.tile_pool(name="bcp", bufs=4))
            ytp = p2.enter_context(tc.tile_pool(name="ytp", bufs=2))
            outp = p2.enter_context(tc.tile_pool(name="outp", bufs=2))
            sps = p2.enter_context(tc.tile_pool(name="sps", bufs=3,
                                                space="PSUM"))
            yps = p2.enter_context(tc.tile_pool(name="yps", bufs=4,
                                                space="PSUM"))
            pps = p2.enter_context(tc.tile_pool(name="pps", bufs=1,
                                                space="PSUM"))

            wp_t = []
            for j in range(4):
                t = wppool.tile([128, N], F32R, name=f"wp{j}", tag=f"wp{j}")
                nc.sync.dma_start(out=t, in_=wp[j * 128:(j + 1) * 128, :])
                wp_t.append(t)

            for qc in range(NQC):
                qcol = qc * 512
                nkb = 4 * qc + 4          # k blocks for this q-chunk
                yt_sb = [ytp.tile([128, 512], F32R, name=f"yt{qc}_{hp}",
                                  tag=f"yt{hp}") for hp in range(4)]
                for hp in range(4):
                    y_ps = [yps.tile([128, 512], F32,
                                     name=f"y_ps{qc}_{hp}_{h01}", tag="y")
                            for h01 in range(2)]
                    def emit_avs(at_l, kb):
                        for h01 in range(2):
                            h = hp * 2 + h01
                            nc.tensor.matmul(
                                y_ps[h01][0:65, :],
                                vt[kb][:, h, :],
                                at_l[h01],
                                start=(kb == 0),
                                stop=(kb == nkb - 1))

                    pend = None
                    for kb in range(nkb):
                        s_l = []
                        for h01 in range(2):
                            hb = h01 * 64
                            s_ps = sps.tile([128, 512], F32,
                                            name=f"s{qc}_{hp}_{h01}_{kb}",
                                            tag="s")
                            nc.tensor.matmul(
                                s_ps,
                                kTt[hp][hb:hb + 64,
                                        kb * 128:(kb + 1) * 128],
                                qT[hp][hb:hb + 64, qcol:qcol + 512],
                                start=True, stop=True)
                            s_l.append(s_ps)
                        at_l = []
                        for h01 in range(2):
                            attT = attTp.tile([128, 512], BF16,
                                              name=f"attT{qc}_{hp}_{h01}_{kb}",
                                              tag="attT")
                            nc.scalar.activation(out=attT, in_=s_l[h01],
                                                 func=AF.Exp, scale=0.125)
                            if kb >= 4 * qc:
                                nc.vector.tensor_mul(attT, attT,
                                                     mt_sb[kb - 4 * qc])
                            at_l.append(attT)
                        if pend is not None:
                            emit_avs(*pend)
                        pend = (at_l, kb)
                    emit_avs(*pend)
                    for h01 in range(2):
                        hb = h01 * 64
                        srow = smallp.tile([1, 512], F32,
                                           name=f"srow{qc}_{hp}_{h01}",
                                           tag="srow")
                        nc.scalar.copy(srow, y_ps[h01][64:65, :])
                        ystg = bcp.tile([64, 512], F32,
                                        name=f"ystg{qc}_{hp}_{h01}",
                                        tag="ystg")
                        nc.vector.tensor_copy(ystg, y_ps[h01][0:64, :])
                        bc = bcp.tile([64, 512], F32,
                                      name=f"bc{qc}_{hp}_{h01}", tag="bc")
                        nc.gpsimd.partition_broadcast(bc, srow)
                        nc.vector.reciprocal_approx_fast(out=bc, in_=bc)
                        nc.vector.tensor_mul(yt_sb[hp][hb:hb + 64, :],
                                             ystg, bc)
                # projection for the 4 q-blocks of this chunk
                for qb in range(4):
                    o_sb = outp.tile([128, N], F32, name=f"o{qc}_{qb}",
                                     tag="o")
                    for nch in range(2):
                        p_ps = pps.tile([128, 512], F32,
                                        name=f"p{qc}_{qb}_{nch}", tag="p")
                        for hp in range(4):
                            nc.tensor.matmul(
                                p_ps,
                                yt_sb[hp][:, qb * 128:(qb + 1) * 128],
                                wp_t[hp][:, nch * 512:(nch + 1) * 512],
                                start=(hp == 0), stop=(hp == 3))
                        nc.vector.tensor_copy(
                            o_sb[:, nch * 512:(nch + 1) * 512], p_ps)
                    nc.sync.dma_start(
                        out=out[qcol + qb * 128:qcol + (qb + 1) * 128, :],
                        in_=o_sb)

    nc.compile()
    _CACHE["nc"] = nc
    return nc


def kernel(x, W_attn, b_attn, W_proj, b_proj):
    global _last_results
    nc = _build()

    x = np.asarray(x, dtype=np.float32)
    W_attn = np.asarray(W_attn, dtype=np.float32)
    b_attn = np.asarray(b_attn, dtype=np.float32)
    W_proj = np.asarray(W_proj, dtype=np.float32)
    b_proj = np.asarray(b_proj, dtype=np.float32)

    kk = np.arange(128)[:, None]
    qq = np.arange(512)[None, :]
    maskt_np = np.stack([(qq >= m * 128 + kk) for m in range(4)]).astype(
        ml_dtypes.bfloat16)

    in_maps = []
    for c in range(NCORES):
        b, hg = divmod(c, 2)
        s = hg * HG
        xT_c = np.ascontiguousarray(x[b].T).astype(ml_dtypes.bfloat16)
        wqk_c = np.ascontiguousarray(
            np.concatenate([W_attn[:, s:s + HG],
                            W_attn[:, N + s:N + s + HG]],
                           axis=1)).astype(ml_dtypes.bfloat16)
        wv_c = np.ascontiguousarray(W_attn[:, 2 * N + s:2 * N + s + HG]).astype(ml_dtypes.bfloat16)
        wp_c = np.ascontiguousarray(W_proj[s:s + HG, :])
        bqk_c = np.ascontiguousarray(
            np.concatenate([b_attn[s:s + HG],
                            b_attn[N + s:N + s + HG]]).reshape(2 * HG, 1))
        in_maps.append({
            "xT": xT_c, "wqk": wqk_c, "wv": wv_c, "wp": wp_c,
            "bqk": bqk_c, "maskt": maskt_np,
        })

    res = run_bass_kernel_spmd(nc, in_maps, list(range(NCORES)))
    _last_results = res
    outs = [res.results[c]["out"] for c in range(NCORES)]
    # v-bias: softmax rows sum to 1, so att @ (xWv + bv) = att @ (xWv) + bv;
    # its projection (bv @ W_proj) plus b_proj are added here, exactly.
    bv = b_attn[2 * N:3 * N]
    extra = bv @ W_proj + b_proj
    y = np.stack([outs[2 * b] + outs[2 * b + 1] for b in range(B)])
    return (y + extra[None, None, :]).astype(np.float32)



# revision 6
# speedup vs baseline: 1.3663x; 1.3663x over previous
"""Multi-head causal attention (B=4, T=2048, N=1024, H=16) on 8 TRN2 NeuronCores.

Sharding: core c = (batch b = c//2, head-group hg = c%2). Each core computes
full-T causal attention for its 8 heads of its batch, plus the partial output
projection for its head rows. Host sums the two head-group partials per batch
and adds b_proj (and the v-bias contribution, which is exact because softmax
rows sum to 1).

Per-core kernel, one globally-scheduled instruction stream (no phase
barriers) so the Tile scheduler can fill ACT-paced attention gaps with
qkv-projection and output-projection matmuls:

  qkv chunk tck (t-cols tck*512):  qkT[j,t] = Wqk.T @ x.T (bias added on
  DVE), v[t,d] = x @ Wv stored bf16 as [64 v | 1] x 8 heads.

  attention (qc, hp): scores TRANSPOSED [k-part, q-free]; the two heads of
  the pair run CONCURRENTLY in the PE array via row-tiling (lhsT at
  partitions 0:64 / 64:128 -> tile_position (0,0)/(64,0)), writing the two
  halves of one [128, 2, 512] PSUM tile.  One fused 1024-wide exp (ACT,
  scale=1/8) covers both heads; diagonal blocks are column-trimmed (the
  fully-masked prefix is neither computed nor exp'd) and only the 128-col
  boundary block is mask-multiplied (bf16 DVE).  The av matmul
  lhsT = v[128,65] (64 v cols + ones) accumulates y.T AND the softmax row
  sums into PSUM [65, 512].  Normalize: DVE reciprocal of the sums row,
  gpsimd partition-broadcast, DVE multiply into the projection-ready yT
  tile.  Projection: yT.T @ Wp in f32r, DMA'd straight from PSUM.

Emission order qkv(0), att(0), qkv(1), proj(0), att(1), ... lets the
dependency scheduler overlap everything; PE stream work (~496k cycles) is
the target critical path.
"""
import numpy as np
import ml_dtypes
from contextlib import ExitStack

import concourse.bass as bass
import concourse.mybir as mybir
from concourse import bacc
from concourse import bass_utils as _bu
from concourse.bass_utils import run_bass_kernel_spmd
from concourse.tile import TileContext


F32 = mybir.dt.float32
F32R = mybir.dt.float32r
BF16 = mybir.dt.bfloat16
AF = mybir.ActivationFunctionType

B, T, N, H = 4, 2048, 1024, 16
Dh = 64
HG = 512            # head-group width per core (8 heads x 64)
NCORES = 8
KT = N // 128       # 8 contraction tiles for qkv
NQC = T // 512      # 4 q-chunks
NVT = T // 128      # 16 v tiles / k blocks

_CACHE = {}
_last_results = None


def _build():
    if "nc" in _CACHE:
        return _CACHE["nc"]

    nc = bacc.Bacc("TRN2", target_bir_lowering=False)

    xT = nc.declare_dram_parameter("xT", [N, T], BF16, isOutput=False)
    wqk = nc.declare_dram_parameter("wqk", [N, 2 * HG], BF16, isOutput=False)
    wv = nc.declare_dram_parameter("wv", [N, HG], BF16, isOutput=False)
    wp = nc.declare_dram_parameter("wp", [HG, N], F32R, isOutput=False)
    bqk = nc.declare_dram_parameter("bqk", [2 * HG, 1], F32, isOutput=False)
    tri = nc.declare_dram_parameter("tri", [128, 128], BF16, isOutput=False)
    out = nc.declare_dram_parameter("out", [T, N], F32, isOutput=True)

    with TileContext(nc) as tc, ExitStack() as st:
        consts = st.enter_context(tc.tile_pool(name="consts", bufs=1))
        wpool = st.enter_context(tc.tile_pool(name="wpool", bufs=1))
        xpool = st.enter_context(tc.tile_pool(name="xpool", bufs=2))
        resid = st.enter_context(tc.tile_pool(name="resid", bufs=1))
        attp = st.enter_context(tc.tile_pool(name="attp", bufs=3))
        smallp = st.enter_context(tc.tile_pool(name="smallp", bufs=2))
        ytp = st.enter_context(tc.tile_pool(name="ytp", bufs=2))
        outp = st.enter_context(tc.tile_pool(name="outp", bufs=2))
        spool = st.enter_context(tc.tile_pool(name="spool", bufs=2,
                                              space="PSUM"))
        ypool = st.enter_context(tc.tile_pool(name="ypool", bufs=1,
                                              space="PSUM"))
        fillps = st.enter_context(tc.tile_pool(name="fillps", bufs=2,
                                               space="PSUM"))

        tri_sb = consts.tile([128, 128], BF16, name="tri_sb", tag="tri")
        nc.sync.dma_start(out=tri_sb, in_=tri[:, :])
        bqk_sb = []
        for j in range(8):
            t = consts.tile([128, 1], F32, name=f"bqk_sb{j}", tag=f"bqk{j}")
            nc.sync.dma_start(out=t, in_=bqk[j * 128:(j + 1) * 128, :])
            bqk_sb.append(t)

        wqk_t = []
        wv_t = []
        wp_t = []
        for k in range(KT):
            t = wpool.tile([128, 2 * HG], BF16, name=f"wqk{k}", tag=f"wqk{k}")
            nc.sync.dma_start(out=t, in_=wqk[k * 128:(k + 1) * 128, :])
            wqk_t.append(t)
            t = wpool.tile([128, HG], BF16, name=f"wv{k}", tag=f"wv{k}")
            nc.sync.dma_start(out=t, in_=wv[k * 128:(k + 1) * 128, :])
            wv_t.append(t)
        for j in range(4):
            t = wpool.tile([128, N], F32R, name=f"wp{j}", tag=f"wp{j}")
            nc.sync.dma_start(out=t, in_=wp[j * 128:(j + 1) * 128, :])
            wp_t.append(t)

        # residents: qT/kT per head-pair tile [128, T]; v per 128-row chunk,
        # bf16, layout [64 v-cols + ones] x 8 heads = 520 cols
        qT = [resid.tile([128, T], BF16, name=f"qT{j}", tag=f"qT{j}")
              for j in range(4)]
        kTt = [resid.tile([128, T], BF16, name=f"kT{j}", tag=f"kT{j}")
               for j in range(4)]
        vt = [resid.tile([128, 8, 65], BF16, name=f"v{m}", tag=f"v{m}")
              for m in range(NVT)]

        def emit_qkv(tck):
            c0 = tck * 512
            xt = []
            for k in range(KT):
                t = xpool.tile([128, 512], BF16, name=f"xt{tck}_{k}",
                               tag=f"x{k}")
                nc.sync.dma_start(out=t, in_=xT[k * 128:(k + 1) * 128,
                                                c0:c0 + 512])
                xt.append(t)
            # qkT [j, t] : lhsT = wqk tile cols, rhs = xT.  Order q0,k0,
            # q1,k1,... so head-pair hp's tensors complete first.
            for jc in (0, 4, 1, 5, 2, 6, 3, 7):
                ps = fillps.tile([128, 512], F32, name=f"qk_ps{tck}_{jc}",
                                 tag="fill")
                for k in range(KT):
                    nc.tensor.matmul(
                        ps,
                        wqk_t[k][:, jc * 128:(jc + 1) * 128],
                        xt[k],
                        start=(k == 0), stop=(k == KT - 1))
                dst = (qT[jc] if jc < 4 else kTt[jc - 4])
                nc.vector.tensor_scalar_add(dst[:, c0:c0 + 512], ps,
                                            bqk_sb[jc])
            # v [t, d] : lhsT = xT tile cols, rhs = wv
            for mc in range(4):
                ps = fillps.tile([128, 512], F32, name=f"v_ps{tck}_{mc}",
                                 tag="fill")
                for k in range(KT):
                    nc.tensor.matmul(
                        ps,
                        xt[k][:, mc * 128:(mc + 1) * 128],
                        wv_t[k],
                        start=(k == 0), stop=(k == KT - 1))
                dst = vt[tck * 4 + mc]
                nc.vector.tensor_copy(
                    dst[:, :, 0:64],
                    ps.rearrange("p (h c) -> p h c", c=64))
                nc.vector.memset(dst[:, :, 64:65], 1.0)

        def emit_att(qc):
            qcol = qc * 512
            nkb = 4 * qc + 4
            yts = []
            for hp in range(4):
                y_ps = [ypool.tile([65, 512], F32,
                                   name=f"y_ps{qc}_{hp}_{h01}",
                                   tag=f"y{h01}")
                        for h01 in range(2)]
                for kb in range(nkb):
                    m = kb - 4 * qc          # >= 0 on diagonal blocks
                    lo = m * 128 if m > 0 else 0
                    s_ps = spool.tile([128, 2, 512], F32,
                                      name=f"s{qc}_{hp}_{kb}", tag="s")
                    for h01 in range(2):
                        hb = h01 * 64
                        nc.tensor.matmul(
                            s_ps[:, h01, lo:512],
                            kTt[hp][hb:hb + 64,
                                    kb * 128:(kb + 1) * 128],
                            qT[hp][hb:hb + 64, qcol + lo:qcol + 512],
                            start=True, stop=True)
                    attT = attp.tile([128, 2, 512], BF16,
                                     name=f"attT{qc}_{hp}_{kb}", tag="attT")
                    nc.scalar.activation(out=attT[:, :, lo:512],
                                         in_=s_ps[:, :, lo:512],
                                         func=AF.Exp, scale=0.125)
                    if m >= 0:
                        if lo > 0:
                            nc.vector.memset(attT[:, :, 0:lo], 0.0)
                        for h01 in range(2):
                            nc.vector.tensor_mul(attT[:, h01, lo:lo + 128],
                                                 attT[:, h01, lo:lo + 128],
                                                 tri_sb)
                    for h01 in range(2):
                        nc.tensor.matmul(
                            y_ps[h01],
                            vt[kb][:, 2 * hp + h01, :],
                            attT[:, h01, :],
                            start=(kb == 0), stop=(kb == nkb - 1))
                yt = ytp.tile([128, 512], F32R, name=f"yt{qc}_{hp}",
                              tag=f"yt{hp}")
                for h01 in range(2):
                    srow = smallp.tile([1, 512], F32,
                                       name=f"srow{qc}_{hp}_{h01}",
                                       tag=f"srow{h01}")
                    nc.scalar.copy(srow, y_ps[h01][64:65, :])
                    ystg = smallp.tile([64, 512], F32,
                                       name=f"ystg{qc}_{hp}_{h01}",
                                       tag=f"ystg{h01}")
                    nc.vector.tensor_copy(ystg, y_ps[h01][0:64, :])
                    bcst = smallp.tile([64, 512], F32,
                                       name=f"bc{qc}_{hp}_{h01}",
                                       tag=f"bc{h01}")
                    nc.gpsimd.partition_broadcast(bcst, srow)
                    nc.vector.reciprocal_approx_fast(out=bcst, in_=bcst)
                    nc.vector.tensor_mul(
                        yt[h01 * 64:(h01 + 1) * 64, :], ystg, bcst)
                yts.append(yt)
            return yts

        def emit_proj(qc, yts):
            qcol = qc * 512
            for qb in range(4):
                o_sb = outp.tile([128, N], F32, name=f"o{qc}_{qb}", tag="o")
                for nch in range(2):
                    p_ps = fillps.tile([128, 512], F32,
                                       name=f"p{qc}_{qb}_{nch}", tag="fill")
                    for hp in range(4):
                        nc.tensor.matmul(
                            p_ps,
                            yts[hp][:, qb * 128:(qb + 1) * 128],
                            wp_t[hp][:, nch * 512:(nch + 1) * 512],
                            start=(hp == 0), stop=(hp == 3))
                    nc.vector.tensor_copy(
                        o_sb[:, nch * 512:(nch + 1) * 512], p_ps)
                nc.sync.dma_start(
                    out=out[qcol + qb * 128:qcol + (qb + 1) * 128, :],
                    in_=o_sb)

        emit_qkv(0)
        for qc in range(NQC):
            yts = emit_att(qc)
            if qc < NQC - 1:
                emit_qkv(qc + 1)
            emit_proj(qc, yts)

    nc.compile()
    _CACHE["nc"] = nc
    return nc


def kernel(x, W_attn, b_attn, W_proj, b_proj):
    global _last_results
    nc = _build()

    x = np.asarray(x, dtype=np.float32)
    W_attn = np.asarray(W_attn, dtype=np.float32)
    b_attn = np.asarray(b_attn, dtype=np.float32)
    W_proj = np.asarray(W_proj, dtype=np.float32)
    b_proj = np.asarray(b_proj, dtype=np.float32)

    kk = np.arange(128)[:, None]
    jj = np.arange(128)[None, :]
    tri_np = (jj >= kk).astype(ml_dtypes.bfloat16)

    in_maps = []
    for c in range(NCORES):
        b, hg = divmod(c, 2)
        s = hg * HG
        xT_c = np.ascontiguousarray(x[b].T).astype(ml_dtypes.bfloat16)
        wqk_c = np.ascontiguousarray(
            np.concatenate([W_attn[:, s:s + HG],
                            W_attn[:, N + s:N + s + HG]],
                           axis=1)).astype(ml_dtypes.bfloat16)
        wv_c = np.ascontiguousarray(
            W_attn[:, 2 * N + s:2 * N + s + HG]).astype(ml_dtypes.bfloat16)
        wp_c = np.ascontiguousarray(W_proj[s:s + HG, :])
        bqk_c = np.ascontiguousarray(
            np.concatenate([b_attn[s:s + HG],
                            b_attn[N + s:N + s + HG]]).reshape(2 * HG, 1))
        in_maps.append({
            "xT": xT_c, "wqk": wqk_c, "wv": wv_c, "wp": wp_c,
            "bqk": bqk_c, "tri": tri_np,
        })

    res = run_bass_kernel_spmd(nc, in_maps, list(range(NCORES)))
    _last_results = res
    outs = [res.results[c]["out"] for c in range(NCORES)]
    # v-bias: softmax rows sum to 1, so att @ (xWv + bv) = att @ (xWv) + bv;
    # its projection (bv @ W_proj) plus b_proj are added here, exactly.
    bv = b_attn[2 * N:3 * N]
    extra = bv @ W_proj + b_proj
    y = np.stack([outs[2 * b] + outs[2 * b + 1] for b in range(B)])
    return (y + extra[None, None, :]).astype(np.float32)
